# revision 32
# baseline (speedup 1.0000x reference)
"""Trainium2 Bass kernel for nn_LocalAttention (T=4096, B=32, H=256, L=512, K=32).

Sharding: data-parallel over batch B across 8 cores (BC=4 batch elements/core).

v2 design (per core):
  - wk in fp8e4 (x64 scaled), single j-major DMA; coeff = lm8 @ wk8 on PE,
    psum -> kernT_sb fp16 via DVE tensor_scalar (x 1/64).  kh ordering
    hc*4096 + k*128 + h so the hc0 half finishes first; relayout via a
    64KB DRAM bounce into (32 k, 2 hc, 4 b, 128 h) conv weights.
  - conv: per (b,hc): 1 LDW + 8 matmuls (K=32, N=512) into (128,1024)
    psum tiles; DVE adds enc (fp16) -> hid fp16; one big ACT tanh
    (128,4096) with per-partition glob bias -> tan fp16.
  - score: ws stationary (P=1), tan streamed N=512, M=1 outputs packed
    4-wide into psum rows {0,32,64,96} via tile_position col groups;
    mask/bs added with one (4,512)-rhs matmul; ACT exp with accum_out
    gives row sums; denominators via selector matmul; normalize on GPSIMD.
"""

import os
import sys

import numpy as np

if "/opt/trn_rl_repo" not in sys.path:
    sys.path.insert(0, "/opt/trn_rl_repo")

import ml_dtypes

T, B, H, L, K = 4096, 32, 256, 512, 32
NCORES = 8
BC = B // NCORES          # 4 batches per core
HCHUNKS = H // 128        # 2
TTILE = 512
NTT = T // TTILE          # 8 t-tiles per (b, hc)
WK_SCALE = 64.0

_CACHE = {}


def _build_program():
    import concourse.bacc as bacc
    import concourse.bass as bass
    import concourse.mybir as mybir
    import concourse.tile as tile
    from contextlib import ExitStack

    dt = mybir.dt
    fp32 = dt.float32
    bf16 = dt.bfloat16
    fp16 = dt.float16
    fp8 = dt.float8e4
    ts = bass.ts

    nc = bacc.Bacc(
        "TRN2",
        target_bir_lowering=False,
        debug=False,
        enable_asserts=False,
        num_devices=NCORES,
    )

    enc = nc.dram_tensor("enc", (BC, HCHUNKS, 128, T), fp16, kind="ExternalInput").ap()
    win = nc.dram_tensor("win", (BC, K, T), fp16, kind="ExternalInput").ap()
    wk8 = nc.dram_tensor("wk8", (128, 16, 4, 512), fp8, kind="ExternalInput").ap()
    lm8 = nc.dram_tensor("lm8", (128, 4, BC), fp8, kind="ExternalInput").ap()
    lmb = nc.dram_tensor("lmb", (128, 4, BC), bf16, kind="ExternalInput").ap()
    wgt = nc.dram_tensor("wgt", (128, 4, H), bf16, kind="ExternalInput").ap()
    bgp = nc.dram_tensor("bgp", (1, H), bf16, kind="ExternalInput").ap()
    ws2 = nc.dram_tensor("ws2", (128, HCHUNKS), fp16, kind="ExternalInput").ap()
    mkc = nc.dram_tensor("mkc", (4, 2 * BC, TTILE), fp8, kind="ExternalInput").ap()
    sel4 = nc.dram_tensor("sel4", (4, 128), fp8, kind="ExternalInput").ap()
    selr = nc.dram_tensor("selr", (128, 1), fp32, kind="ExternalInput").ap()
    att = nc.dram_tensor("att", (4, 2 * BC, TTILE), fp16, kind="ExternalOutput").ap()

    TanhF = mybir.ActivationFunctionType.Tanh
    ExpF = mybir.ActivationFunctionType.Exp
    Add = mybir.AluOpType.add

    with tile.TileContext(nc) as tc, ExitStack() as ctx:
        # ---------- pools ----------
        small_pool = ctx.enter_context(tc.tile_pool(name="small", bufs=1))
        big_sb = ctx.enter_context(tc.tile_pool(name="bigsb", bufs=1))
        wk_pool = ctx.enter_context(tc.tile_pool(name="wkp", bufs=16))
        kern_pool = ctx.enter_context(tc.tile_pool(name="kernp", bufs=1))
        hid_pool = ctx.enter_context(tc.tile_pool(name="hidp", bufs=3))
        tan_pool = ctx.enter_context(tc.tile_pool(name="tanp", bufs=3))
        psum_big = ctx.enter_context(tc.tile_pool(name="psumb", bufs=2, space="PSUM"))
        psum_sp = ctx.enter_context(tc.tile_pool(name="psums", bufs=2, space="PSUM"))
        dram_pool = ctx.enter_context(tc.tile_pool(name="dramp", bufs=1, space="DRAM"))

        # ---------- small input loads (sync queue, before enc/win) ----------
        lm8_sb = small_pool.tile([128, 4, BC], fp8)
        nc.sync.dma_start(lm8_sb[:], lm8)
        lmb_sb = small_pool.tile([128, 4, BC], bf16)
        nc.sync.dma_start(lmb_sb[:], lmb)
        wgt_sb = small_pool.tile([128, 4, H], bf16)
        nc.sync.dma_start(wgt_sb[:], wgt)
        bg_sb = small_pool.tile([1, H], bf16)
        nc.sync.dma_start(bg_sb[:], bgp)
        ws_sb = small_pool.tile([128, HCHUNKS], fp16)
        nc.sync.dma_start(ws_sb[:], ws2)
        msk_sb = small_pool.tile([4, 2 * BC, TTILE], fp8)
        nc.sync.dma_start(msk_sb[:], mkc)
        sel4_sb = small_pool.tile([4, 128], fp8)
        nc.sync.dma_start(sel4_sb[:], sel4)
        selr_sb = small_pool.tile([128, 1], fp32)
        nc.sync.dma_start(selr_sb[:], selr)

        one_b = small_pool.tile([1, BC], bf16)
        nc.vector.memset(one_b[:], 1.0)
        ones1x128 = small_pool.tile([1, 128], fp32)
        nc.vector.memset(ones1x128[:], 1.0)

        # PE warmup filler: keep HAM busy from t~1.5us until the wk stream
        # arrives (~10.5us) so the coeff phase runs at 2.4 GHz
        warm_sb = small_pool.tile([128, 128], bf16)
        nc.vector.memset(warm_sb[:], 0.0)
        wps = psum_sp.tile([64, 128], fp32, tag="sp", name="wps")
        for _ in range(90):
            nc.tensor.matmul(
                wps[:], warm_sb[:, 0:64], warm_sb[:], start=True, stop=True
            )


        # ---------- big input loads ----------
        # wk8 j-major in 16 small chunks: PE never starves, HAM warms early
        wk_tiles = []
        for jc in range(16):
            wkt = wk_pool.tile([128, 1, 4, 512], fp8, tag="wk", name=f"wk{jc}")
            nc.gpsimd.dma_start(wkt[:], wk8[:, jc : jc + 1, :, :])
            wk_tiles.append(wkt)
        # win: all 4 b packed at rows {0,32,64,96}; on the priority queue
        winA = big_sb.tile([128, T], fp16)
        for b in range(BC):
            nc.gpsimd.dma_start(winA[32 * b : 32 * b + 32, :], win[b, :, :])
        enc_sb = big_sb.tile([128, BC, HCHUNKS, T], fp16)
        # gpsimd: first 4 units' enc; sync: late units (after wk/win clear)
        enc_order = [(0, 0), (0, 1), (1, 0), (1, 1), (2, 0), (2, 1), (3, 0), (3, 1)]
        for n, (b, hc) in enumerate(enc_order):
            nc.gpsimd.dma_start(enc_sb[:, b, hc, :], enc[b, hc, :, :])

        # ---------- persistent sbuf ----------
        scr = dram_pool.tile([BC, K * H], fp16)
        coef_sb = big_sb.tile([128, HCHUNKS, 128], fp16)  # [32*b + k, hc, h]
        exp_sb = big_sb.tile([128, 2 * BC, TTILE], fp16)
        acc_sb = small_pool.tile([128, 2 * BC], fp32)
        att_sb = big_sb.tile([128, 2 * BC, TTILE], fp16)
        glob_sb = small_pool.tile([128, HCHUNKS, BC], fp32)
        dsum_sb = small_pool.tile([1, 2 * BC], fp32)
        den_sb = small_pool.tile([1, BC], fp32)
        rec_sb = small_pool.tile([1, BC], fp32)
        recb_sb = small_pool.tile([128, BC], fp32)

        def coeff_chunk(jlist, kernT_sb):
            for j in jlist:
                cps = psum_big.tile([BC, 512], fp32, tag="conv", name="cps")
                for lc in range(4):
                    nc.tensor.matmul(
                        cps[:],
                        lm8_sb[:, lc, :],
                        wk_tiles[j][:, 0, lc, :],
                        start=(lc == 0),
                        stop=(lc == 3),
                    )
                nc.vector.tensor_scalar_mul(
                    kernT_sb[:, ts(j % 8, 512)], cps[:], 1.0 / WK_SCALE
                )

        def relayout(hcc, kernT_sb):
            nc.sync.dma_start(scr[:, ts(hcc, 4096)], kernT_sb[:])
            for b in range(BC):
                nc.sync.dma_start(
                    coef_sb[32 * b : 32 * b + 32, hcc, :],
                    scr[b, ts(hcc, 4096)].rearrange("(k h) -> k h", k=K),
                )

        def glob_phase():
            for hc in range(HCHUNKS):
                gps = psum_sp.tile([128, BC], fp32, tag="sp", name="gps")
                for lc in range(4):
                    nc.tensor.matmul(
                        gps[:],
                        wgt_sb[:, lc, ts(hc, 128)],
                        lmb_sb[:, lc, :],
                        start=(lc == 0),
                        stop=False,
                    )
                nc.tensor.matmul(
                    gps[:], bg_sb[:, ts(hc, 128)], one_b[:], start=False, stop=True
                )
                nc.scalar.copy(glob_sb[:, hc, :], gps[:])

        def conv_unit(b, hc):
            """conv + enc add + tanh for one (b, hc); returns tan tile."""
            lhsT = coef_sb[32 * b : 32 * b + 32, hc, :]
            base = 32 * b
            hid = hid_pool.tile([128, T], fp16, tag="hid")
            # 8 tt-slices grouped as 3+3+2 psum tiles (3-bank tiles cut
            # DVE per-op overhead; pool bufs=2 x 3 banks = 6 banks)
            off = 0
            for ntile in (3, 3, 2):
                w = ntile * 512
                cpsum = psum_big.tile([128, 1536], fp32, tag="conv", name="cpsum")
                for q in range(ntile):
                    nc.tensor.matmul(
                        cpsum[:, ts(q, 512)],
                        lhsT,
                        winA[base : base + 32, ts(off + q, 512)],
                        start=True,
                        stop=True,
                        tile_position=(base, 0),
                    )
                nc.vector.tensor_tensor(
                    hid[:, off * 512 : off * 512 + w],
                    cpsum[:, 0:w],
                    enc_sb[:, b, hc, off * 512 : off * 512 + w],
                    Add,
                )
                off += ntile
            tan = tan_pool.tile([128, T], fp16, tag="tan")
            nc.scalar.activation(
                tan[:], hid[:], TanhF, bias=glob_sb[:, hc, b : b + 1], scale=1.0
            )
            return tan

        def score_unit(b, tans):
            """score matmuls + exp for both s halves of batch b."""
            for s in range(2):
                sp = psum_sp.tile([128, TTILE], fp32, tag="sp", name="sp")
                nc.tensor.matmul(
                    sp[:],
                    sel4_sb[:],
                    msk_sb[:, s * BC + b, :],
                    start=True,
                    stop=False,
                    skip_group_check=True,
                )
                for j in range(4):
                    for hc in range(HCHUNKS):
                        nc.tensor.matmul(
                            sp[32 * j : 32 * j + 1, :],
                            ws_sb[:, hc : hc + 1],
                            tans[hc][:, ts(4 * s + j, 512)],
                            start=False,
                            stop=(j == 3 and hc == HCHUNKS - 1),
                            tile_position=(0, 32 * j),
                            skip_group_check=True,
                        )
                col = s * BC + b
                nc.scalar.activation(
                    exp_sb[:, col, :],
                    sp[:],
                    ExpF,
                    bias=0.0,
                    scale=1.0,
                    accum_out=acc_sb[:, col : col + 1],
                )

        def softmax_tail():
            dps = psum_sp.tile([1, 2 * BC], fp32, tag="sp", name="dps")
            nc.tensor.matmul(dps[:], selr_sb[:], acc_sb[:], start=True, stop=True)
            nc.scalar.copy(dsum_sb[:], dps[:])
            nc.vector.tensor_tensor(
                den_sb[:], dsum_sb[:, 0:BC], dsum_sb[:, BC : 2 * BC], Add
            )
            nc.vector.reciprocal(rec_sb[:], den_sb[:])
            bps = psum_sp.tile([128, BC], fp32, tag="sp", name="bps")
            nc.tensor.matmul(bps[:], ones1x128[:], rec_sb[:], start=True, stop=True)
            nc.scalar.copy(recb_sb[:], bps[:])
            for b in range(BC):
                nc.scalar.mul(
                    att_sb[:, b, :], exp_sb[:, b, :], recb_sb[:, b : b + 1]
                )
                nc.vector.tensor_scalar_mul(
                    att_sb[:, BC + b, :], exp_sb[:, BC + b, :], recb_sb[:, b : b + 1]
                )

        # ---- emission order ----
        kernT0 = kern_pool.tile([BC, 8 * 512], fp16, tag="kern", name="kernT0")
        coeff_chunk(range(0, 8), kernT0)
        relayout(0, kernT0)
        glob_phase()

        kernT1 = kern_pool.tile([BC, 8 * 512], fp16, tag="kern", name="kernT1")
        coeff_chunk(range(8, 16), kernT1)
        relayout(1, kernT1)

        for b in range(BC):
            t0 = conv_unit(b, 0)
            t1 = conv_unit(b, 1)
            score_unit(b, [t0, t1])
        softmax_tail()

        for j in range(4):
            nc.scalar.dma_start(att[j : j + 1], att_sb[32 * j : 32 * j + 1, :, :])

    nc.compile()
    return nc


def _get_program():
    if "nc" not in _CACHE:
        _CACHE["nc"] = _build_program()
    return _CACHE["nc"]


def _prep_inputs(encoded_contribution, mask, lm_state, prev_att_weights,
                 Wk, bk, Wg, bg, Ws, bs):
    """Host-side shard + layout prep. Returns list of per-core input dicts."""
    import concourse.mybir as mybir

    f32 = np.float32
    bf16 = ml_dtypes.bfloat16
    f8 = mybir.dt.np(mybir.dt.float8e4)

    enc = np.asarray(encoded_contribution, dtype=f32)
    mask = np.asarray(mask, dtype=f32)
    lm = np.asarray(lm_state, dtype=f32)
    prev = np.asarray(prev_att_weights, dtype=f32)
    Wk = np.asarray(Wk, dtype=f32)
    bk = np.asarray(bk, dtype=f32)
    Wg = np.asarray(Wg, dtype=f32)
    bg = np.asarray(bg, dtype=f32)
    Ws = np.asarray(Ws, dtype=f32)
    bs = np.asarray(bs, dtype=f32)

    # toeplitz windows: win[b, k, t] = prev_pad[b, k + t]
    prev_pad = np.zeros((B, T + K - 1), dtype=f32)
    prev_pad[:, K - 1 :] = prev.T
    win_f32 = np.lib.stride_tricks.sliding_window_view(prev_pad, T, axis=1)

    # fold the conv bias bk into enc: contribution = sum_k win[b,k,t]*bk[h,k]
    if np.any(bk):
        enc = enc + np.einsum(
            "bkt,hk->tbh", win_f32, bk.reshape(H, K), optimize=True
        )

    # enc: (T, B, H) -> (B, H, T) -> (NCORES, BC, HCHUNKS, 128, T) fp16
    enc_t = np.ascontiguousarray(enc.transpose(1, 2, 0).astype(np.float16)).reshape(
        NCORES, BC, HCHUNKS, 128, T
    )
    win_full = win_f32.astype(np.float16).reshape(NCORES, BC, K, T)

    # WkP64[l, kh'] with kh' = hc*4096 + k*128 + h  (Wk row = (hc*128+h)*32 + k)
    wkp = (
        Wk.reshape(HCHUNKS, 128, K, L)       # (hc, h, k, l)
        .transpose(3, 0, 2, 1)               # (l, hc, k, h)
        .reshape(L, K * H)
        * WK_SCALE
    ).astype(f8)
    # dram layout (128 lp, 16 j, 4 lc, 512 c): [lc*128+lp, j*512+c]
    wk8 = np.ascontiguousarray(
        wkp.reshape(4, 128, 16, 512).transpose(1, 2, 0, 3)
    )


    # lm chunks: (128, 4, B)
    lmT = np.ascontiguousarray(lm.T.reshape(4, 128, B).transpose(1, 0, 2))

    # WgT chunks: (128, 4, H)
    wgt = np.ascontiguousarray(Wg.T.reshape(4, 128, H).transpose(1, 0, 2)).astype(bf16)

    bgp = np.ascontiguousarray(bg.reshape(1, H)).astype(bf16)
    ws2 = np.ascontiguousarray(Ws[0].reshape(HCHUNKS, 128).T).astype(np.float16)

    # selector constants
    sel4 = np.zeros((4, 128), dtype=f8)
    for p in range(4):
        sel4[p, 32 * p] = 1.0
    selr = np.zeros((128, 1), dtype=f32)
    selr[::32, 0] = 1.0

    in_maps = []
    for c in range(NCORES):
        m = mask[:, c * BC : (c + 1) * BC] + bs[0]   # (T, BC)
        # mkc[j, b*2+s, c] = m[(4s+j)*512 + c, b]
        mr = m.reshape(2, 4, TTILE, BC)              # (s, j, cc, b)
        mkc = np.ascontiguousarray(
            np.clip(mr.transpose(1, 0, 3, 2).reshape(4, 2 * BC, TTILE), -440.0, 440.0)
        ).astype(f8)
        lmc = np.ascontiguousarray(lmT[:, :, c * BC : (c + 1) * BC])
        in_maps.append(
            {
                "enc": np.ascontiguousarray(enc_t[c]),
                "win": np.ascontiguousarray(win_full[c]),
                "wk8": wk8,
                "lm8": lmc.astype(f8),
                "lmb": lmc.astype(bf16),
                "wgt": wgt,
                "bgp": bgp,
                "ws2": ws2,
                "mkc": mkc,
                "sel4": sel4,
                "selr": selr,
            }
        )
    return in_maps


def _assemble_output(per_core):
    out = np.empty((T, B), dtype=np.float32)
    for c in range(NCORES):
        A = np.asarray(per_core[c], dtype=np.float32)   # (4, 2*BC, 512)
        # A[j, s*BC+b, cc] = att[t=(4s+j)*512+cc, c*BC+b]
        blk = A.reshape(4, 2, BC, TTILE).transpose(1, 0, 3, 2).reshape(T, BC)
        out[:, c * BC : (c + 1) * BC] = blk
    return out


def kernel(**inputs):
    from concourse.bass_utils import run_bass_kernel_spmd

    in_maps = _prep_inputs(**inputs)
    nc = _get_program()
    trace = bool(os.environ.get("BASS_TRACE"))
    res = run_bass_kernel_spmd(nc, in_maps, list(range(NCORES)), trace=trace)
    _CACHE["last_results"] = res
    return _assemble_output([r["att"] for r in res.results])


# revision 33
# speedup vs baseline: 1.0567x; 1.0567x over previous
"""Trainium2 Bass kernel for nn_LocalAttention (T=4096, B=32, H=256, L=512, K=32).

Sharding: data-parallel over batch B across 8 cores (BC=4 batch elements/core).

v2 design (per core):
  - wk in fp8e4 (x64 scaled), single j-major DMA; coeff = lm8 @ wk8 on PE,
    psum -> kernT_sb fp16 via DVE tensor_scalar (x 1/64).  kh ordering
    hc*4096 + k*128 + h so the hc0 half finishes first; relayout via a
    64KB DRAM bounce into (32 k, 2 hc, 4 b, 128 h) conv weights.
  - conv: per (b,hc): 1 LDW + 8 matmuls (K=32, N=512) into (128,1024)
    psum tiles; DVE adds enc (fp16) -> hid fp16; one big ACT tanh
    (128,4096) with per-partition glob bias -> tan fp16.
  - score: ws stationary (P=1), tan streamed N=512, M=1 outputs packed
    4-wide into psum rows {0,32,64,96} via tile_position col groups;
    mask/bs added with one (4,512)-rhs matmul; ACT exp with accum_out
    gives row sums; denominators via selector matmul; normalize on GPSIMD.
"""

import os
import sys

import numpy as np

if "/opt/trn_rl_repo" not in sys.path:
    sys.path.insert(0, "/opt/trn_rl_repo")

import ml_dtypes

T, B, H, L, K = 4096, 32, 256, 512, 32
NCORES = 8
BC = B // NCORES          # 4 batches per core
HCHUNKS = H // 128        # 2
TTILE = 512
NTT = T // TTILE          # 8 t-tiles per (b, hc)
WK_SCALE = 64.0

_CACHE = {}


def _build_program():
    import concourse.bacc as bacc
    import concourse.bass as bass
    import concourse.mybir as mybir
    import concourse.tile as tile
    from contextlib import ExitStack

    dt = mybir.dt
    fp32 = dt.float32
    bf16 = dt.bfloat16
    fp16 = dt.float16
    fp8 = dt.float8e4
    ts = bass.ts

    nc = bacc.Bacc(
        "TRN2",
        target_bir_lowering=False,
        debug=False,
        enable_asserts=False,
        num_devices=NCORES,
    )

    enc = nc.dram_tensor("enc", (BC, HCHUNKS, 128, T), fp16, kind="ExternalInput").ap()
    win = nc.dram_tensor("win", (BC, K, T), fp16, kind="ExternalInput").ap()
    wk8 = nc.dram_tensor("wk8", (128, 16, 4, 512), fp8, kind="ExternalInput").ap()
    lm8 = nc.dram_tensor("lm8", (128, 4, BC), fp8, kind="ExternalInput").ap()
    lmb = nc.dram_tensor("lmb", (128, 4, BC), bf16, kind="ExternalInput").ap()
    wgt = nc.dram_tensor("wgt", (128, 4, H), bf16, kind="ExternalInput").ap()
    bgp = nc.dram_tensor("bgp", (1, H), bf16, kind="ExternalInput").ap()
    ws2 = nc.dram_tensor("ws2", (128, HCHUNKS), fp16, kind="ExternalInput").ap()
    mkc = nc.dram_tensor("mkc", (4, 2 * BC, TTILE), fp8, kind="ExternalInput").ap()
    sel4 = nc.dram_tensor("sel4", (4, 128), fp8, kind="ExternalInput").ap()
    selr = nc.dram_tensor("selr", (128, 1), fp32, kind="ExternalInput").ap()
    att = nc.dram_tensor("att", (4, 2 * BC, TTILE), fp16, kind="ExternalOutput").ap()

    TanhF = mybir.ActivationFunctionType.Tanh
    ExpF = mybir.ActivationFunctionType.Exp
    Add = mybir.AluOpType.add

    with tile.TileContext(nc) as tc, ExitStack() as ctx:
        # ---------- pools ----------
        small_pool = ctx.enter_context(tc.tile_pool(name="small", bufs=1))
        big_sb = ctx.enter_context(tc.tile_pool(name="bigsb", bufs=1))
        wk_pool = ctx.enter_context(tc.tile_pool(name="wkp", bufs=16))
        kern_pool = ctx.enter_context(tc.tile_pool(name="kernp", bufs=1))
        hid_pool = ctx.enter_context(tc.tile_pool(name="hidp", bufs=3))
        tan_pool = ctx.enter_context(tc.tile_pool(name="tanp", bufs=3))
        psum_big = ctx.enter_context(tc.tile_pool(name="psumb", bufs=3, space="PSUM"))
        psum_sp = ctx.enter_context(tc.tile_pool(name="psums", bufs=2, space="PSUM"))
        dram_pool = ctx.enter_context(tc.tile_pool(name="dramp", bufs=1, space="DRAM"))

        # ---------- small input loads (sync queue, before enc/win) ----------
        lm8_sb = small_pool.tile([128, 4, BC], fp8)
        nc.sync.dma_start(lm8_sb[:], lm8)
        lmb_sb = small_pool.tile([128, 4, BC], bf16)
        nc.sync.dma_start(lmb_sb[:], lmb)
        wgt_sb = small_pool.tile([128, 4, H], bf16)
        nc.sync.dma_start(wgt_sb[:], wgt)
        bg_sb = small_pool.tile([1, H], bf16)
        nc.sync.dma_start(bg_sb[:], bgp)
        ws_sb = small_pool.tile([128, HCHUNKS], fp16)
        nc.sync.dma_start(ws_sb[:], ws2)
        msk_sb = small_pool.tile([4, 2 * BC, TTILE], fp8)
        nc.sync.dma_start(msk_sb[:], mkc)
        sel4_sb = small_pool.tile([4, 128], fp8)
        nc.sync.dma_start(sel4_sb[:], sel4)
        selr_sb = small_pool.tile([128, 1], fp32)
        nc.sync.dma_start(selr_sb[:], selr)

        one_b = small_pool.tile([1, BC], bf16)
        nc.vector.memset(one_b[:], 1.0)
        ones1x128 = small_pool.tile([1, 128], fp32)
        nc.vector.memset(ones1x128[:], 1.0)

        # PE warmup filler: keep HAM busy from t~1.5us until the wk stream
        # arrives (~10.5us) so the coeff phase runs at 2.4 GHz
        warm_sb = small_pool.tile([128, 128], bf16)
        nc.vector.memset(warm_sb[:], 0.0)
        wps = psum_sp.tile([64, 128], fp32, tag="sp", name="wps")
        for _ in range(120):
            nc.tensor.matmul(
                wps[:], warm_sb[:, 0:64], warm_sb[:], start=True, stop=True
            )


        # ---------- big input loads ----------
        # wk8 j-major in 16 small chunks: PE never starves, HAM warms early
        wk_tiles = []
        for jc in range(16):
            wkt = wk_pool.tile([128, 1, 4, 512], fp8, tag="wk", name=f"wk{jc}")
            nc.gpsimd.dma_start(wkt[:], wk8[:, jc : jc + 1, :, :])
            wk_tiles.append(wkt)
        # win: all 4 b packed at rows {0,32,64,96}; on the priority queue
        winA = big_sb.tile([128, T], fp16)
        for b in range(BC):
            nc.gpsimd.dma_start(winA[32 * b : 32 * b + 32, :], win[b, :, :])
        enc_sb = big_sb.tile([128, BC, HCHUNKS, T], fp16)
        # gpsimd: first 4 units' enc; sync: late units (after wk/win clear)
        enc_order = [(0, 0), (0, 1), (1, 0), (1, 1), (2, 0), (2, 1), (3, 0), (3, 1)]
        for n, (b, hc) in enumerate(enc_order):
            nc.gpsimd.dma_start(enc_sb[:, b, hc, :], enc[b, hc, :, :])

        # ---------- persistent sbuf ----------
        scr = dram_pool.tile([BC, K * H], fp16)
        coef_sb = big_sb.tile([128, HCHUNKS, 128], fp16)  # [32*b + k, hc, h]
        exp_sb = big_sb.tile([128, 2 * BC, TTILE], fp16)
        acc_sb = small_pool.tile([128, 2 * BC], fp32)
        att_sb = big_sb.tile([128, 2 * BC, TTILE], fp16)
        glob_sb = small_pool.tile([128, HCHUNKS, BC], fp32)
        dsum_sb = small_pool.tile([1, 2 * BC], fp32)
        den_sb = small_pool.tile([1, BC], fp32)
        rec_sb = small_pool.tile([1, BC], fp32)
        recb_sb = small_pool.tile([128, BC], fp32)

        def coeff_chunk(jlist, kernT_sb):
            for j in jlist:
                cps = psum_big.tile([BC, 512], fp32, tag="conv", name="cps")
                for lc in range(4):
                    nc.tensor.matmul(
                        cps[:],
                        lm8_sb[:, lc, :],
                        wk_tiles[j][:, 0, lc, :],
                        start=(lc == 0),
                        stop=(lc == 3),
                    )
                nc.vector.tensor_scalar_mul(
                    kernT_sb[:, ts(j % 8, 512)], cps[:], 1.0 / WK_SCALE
                )

        def relayout(hcc, kernT_sb):
            nc.sync.dma_start(scr[:, ts(hcc, 4096)], kernT_sb[:])
            for b in range(BC):
                nc.sync.dma_start(
                    coef_sb[32 * b : 32 * b + 32, hcc, :],
                    scr[b, ts(hcc, 4096)].rearrange("(k h) -> k h", k=K),
                )

        def glob_phase():
            for hc in range(HCHUNKS):
                gps = psum_sp.tile([128, BC], fp32, tag="sp", name="gps")
                for lc in range(4):
                    nc.tensor.matmul(
                        gps[:],
                        wgt_sb[:, lc, ts(hc, 128)],
                        lmb_sb[:, lc, :],
                        start=(lc == 0),
                        stop=False,
                    )
                nc.tensor.matmul(
                    gps[:], bg_sb[:, ts(hc, 128)], one_b[:], start=False, stop=True
                )
                nc.scalar.copy(glob_sb[:, hc, :], gps[:])

        def conv_unit(b, hc):
            """conv + enc add + tanh for one (b, hc); returns tan tile."""
            lhsT = coef_sb[32 * b : 32 * b + 32, hc, :]
            base = 32 * b
            hid = hid_pool.tile([128, T], fp16, tag="hid")
            for th in range(4):
                cpsum = psum_big.tile([128, 1024], fp32, tag="conv", name="cpsum")
                for q in range(2):
                    tt = th * 2 + q
                    nc.tensor.matmul(
                        cpsum[:, ts(q, 512)],
                        lhsT,
                        winA[base : base + 32, ts(tt, 512)],
                        start=True,
                        stop=True,
                        tile_position=(base, 0),
                    )
                nc.vector.tensor_tensor(
                    hid[:, ts(th, 1024)],
                    cpsum[:],
                    enc_sb[:, b, hc, ts(th, 1024)],
                    Add,
                )
            tan = tan_pool.tile([128, T], fp16, tag="tan")
            nc.scalar.activation(
                tan[:], hid[:], TanhF, bias=glob_sb[:, hc, b : b + 1], scale=1.0
            )
            return tan

        def score_unit(b, tans):
            """score matmuls + exp for both s halves of batch b."""
            for s in range(2):
                sp = psum_sp.tile([128, TTILE], fp32, tag="sp", name="sp")
                nc.tensor.matmul(
                    sp[:],
                    sel4_sb[:],
                    msk_sb[:, s * BC + b, :],
                    start=True,
                    stop=False,
                    skip_group_check=True,
                )
                for j in range(4):
                    for hc in range(HCHUNKS):
                        nc.tensor.matmul(
                            sp[32 * j : 32 * j + 1, :],
                            ws_sb[:, hc : hc + 1],
                            tans[hc][:, ts(4 * s + j, 512)],
                            start=False,
                            stop=(j == 3 and hc == HCHUNKS - 1),
                            tile_position=(0, 32 * j),
                            skip_group_check=True,
                        )
                col = s * BC + b
                nc.scalar.activation(
                    exp_sb[:, col, :],
                    sp[:],
                    ExpF,
                    bias=0.0,
                    scale=1.0,
                    accum_out=acc_sb[:, col : col + 1],
                )

        def softmax_tail():
            dps = psum_sp.tile([1, 2 * BC], fp32, tag="sp", name="dps")
            nc.tensor.matmul(dps[:], selr_sb[:], acc_sb[:], start=True, stop=True)
            nc.scalar.copy(dsum_sb[:], dps[:])
            nc.vector.tensor_tensor(
                den_sb[:], dsum_sb[:, 0:BC], dsum_sb[:, BC : 2 * BC], Add
            )
            nc.vector.reciprocal(rec_sb[:], den_sb[:])
            bps = psum_sp.tile([128, BC], fp32, tag="sp", name="bps")
            nc.tensor.matmul(bps[:], ones1x128[:], rec_sb[:], start=True, stop=True)
            nc.scalar.copy(recb_sb[:], bps[:])
            for b in range(BC):
                nc.scalar.mul(
                    att_sb[:, b, :], exp_sb[:, b, :], recb_sb[:, b : b + 1]
                )
                nc.vector.tensor_scalar_mul(
                    att_sb[:, BC + b, :], exp_sb[:, BC + b, :], recb_sb[:, b : b + 1]
                )

        # ---- emission order ----
        kernT0 = kern_pool.tile([BC, 8 * 512], fp16, tag="kern", name="kernT0")
        coeff_chunk(range(0, 8), kernT0)
        relayout(0, kernT0)
        glob_phase()

        kernT1 = kern_pool.tile([BC, 8 * 512], fp16, tag="kern", name="kernT1")
        coeff_chunk(range(8, 16), kernT1)
        relayout(1, kernT1)

        for b in range(BC):
            t0 = conv_unit(b, 0)
            t1 = conv_unit(b, 1)
            score_unit(b, [t0, t1])
        softmax_tail()

        for j in range(4):
            nc.scalar.dma_start(att[j : j + 1], att_sb[32 * j : 32 * j + 1, :, :])

    nc.compile()
    return nc


def _get_program():
    if "nc" not in _CACHE:
        _CACHE["nc"] = _build_program()
    return _CACHE["nc"]


def _prep_inputs(encoded_contribution, mask, lm_state, prev_att_weights,
                 Wk, bk, Wg, bg, Ws, bs):
    """Host-side shard + layout prep. Returns list of per-core input dicts."""
    import concourse.mybir as mybir

    f32 = np.float32
    bf16 = ml_dtypes.bfloat16
    f8 = mybir.dt.np(mybir.dt.float8e4)

    enc = np.asarray(encoded_contribution, dtype=f32)
    mask = np.asarray(mask, dtype=f32)
    lm = np.asarray(lm_state, dtype=f32)
    prev = np.asarray(prev_att_weights, dtype=f32)
    Wk = np.asarray(Wk, dtype=f32)
    bk = np.asarray(bk, dtype=f32)
    Wg = np.asarray(Wg, dtype=f32)
    bg = np.asarray(bg, dtype=f32)
    Ws = np.asarray(Ws, dtype=f32)
    bs = np.asarray(bs, dtype=f32)

    # toeplitz windows: win[b, k, t] = prev_pad[b, k + t]
    prev_pad = np.zeros((B, T + K - 1), dtype=f32)
    prev_pad[:, K - 1 :] = prev.T
    win_f32 = np.lib.stride_tricks.sliding_window_view(prev_pad, T, axis=1)

    # fold the conv bias bk into enc: contribution = sum_k win[b,k,t]*bk[h,k]
    if np.any(bk):
        enc = enc + np.einsum(
            "bkt,hk->tbh", win_f32, bk.reshape(H, K), optimize=True
        )

    # enc: (T, B, H) -> (B, H, T) -> (NCORES, BC, HCHUNKS, 128, T) fp16
    enc_t = np.ascontiguousarray(enc.transpose(1, 2, 0).astype(np.float16)).reshape(
        NCORES, BC, HCHUNKS, 128, T
    )
    win_full = win_f32.astype(np.float16).reshape(NCORES, BC, K, T)

    # WkP64[l, kh'] with kh' = hc*4096 + k*128 + h  (Wk row = (hc*128+h)*32 + k)
    wkp = (
        Wk.reshape(HCHUNKS, 128, K, L)       # (hc, h, k, l)
        .transpose(3, 0, 2, 1)               # (l, hc, k, h)
        .reshape(L, K * H)
        * WK_SCALE
    ).astype(f8)
    # dram layout (128 lp, 16 j, 4 lc, 512 c): [lc*128+lp, j*512+c]
    wk8 = np.ascontiguousarray(
        wkp.reshape(4, 128, 16, 512).transpose(1, 2, 0, 3)
    )


    # lm chunks: (128, 4, B)
    lmT = np.ascontiguousarray(lm.T.reshape(4, 128, B).transpose(1, 0, 2))

    # WgT chunks: (128, 4, H)
    wgt = np.ascontiguousarray(Wg.T.reshape(4, 128, H).transpose(1, 0, 2)).astype(bf16)

    bgp = np.ascontiguousarray(bg.reshape(1, H)).astype(bf16)
    ws2 = np.ascontiguousarray(Ws[0].reshape(HCHUNKS, 128).T).astype(np.float16)

    # selector constants
    sel4 = np.zeros((4, 128), dtype=f8)
    for p in range(4):
        sel4[p, 32 * p] = 1.0
    selr = np.zeros((128, 1), dtype=f32)
    selr[::32, 0] = 1.0

    in_maps = []
    for c in range(NCORES):
        m = mask[:, c * BC : (c + 1) * BC] + bs[0]   # (T, BC)
        # mkc[j, b*2+s, c] = m[(4s+j)*512 + c, b]
        mr = m.reshape(2, 4, TTILE, BC)              # (s, j, cc, b)
        mkc = np.ascontiguousarray(
            np.clip(mr.transpose(1, 0, 3, 2).reshape(4, 2 * BC, TTILE), -440.0, 440.0)
        ).astype(f8)
        lmc = np.ascontiguousarray(lmT[:, :, c * BC : (c + 1) * BC])
        in_maps.append(
            {
                "enc": np.ascontiguousarray(enc_t[c]),
                "win": np.ascontiguousarray(win_full[c]),
                "wk8": wk8,
                "lm8": lmc.astype(f8),
                "lmb": lmc.astype(bf16),
                "wgt": wgt,
                "bgp": bgp,
                "ws2": ws2,
                "mkc": mkc,
                "sel4": sel4,
                "selr": selr,
            }
        )
    return in_maps


def _assemble_output(per_core):
    out = np.empty((T, B), dtype=np.float32)
    for c in range(NCORES):
        A = np.asarray(per_core[c], dtype=np.float32)   # (4, 2*BC, 512)
        # A[j, s*BC+b, cc] = att[t=(4s+j)*512+cc, c*BC+b]
        blk = A.reshape(4, 2, BC, TTILE).transpose(1, 0, 3, 2).reshape(T, BC)
        out[:, c * BC : (c + 1) * BC] = blk
    return out


def kernel(**inputs):
    from concourse.bass_utils import run_bass_kernel_spmd

    in_maps = _prep_inputs(**inputs)
    nc = _get_program()
    trace = bool(os.environ.get("BASS_TRACE"))
    res = run_bass_kernel_spmd(nc, in_maps, list(range(NCORES)), trace=trace)
    _CACHE["last_results"] = res
    return _assemble_output([r["att"] for r in res.results])


# revision 34
# speedup vs baseline: 1.0570x; 1.0003x over previous
"""Trainium2 Bass kernel for nn_LocalAttention (T=4096, B=32, H=256, L=512, K=32).

Sharding: data-parallel over batch B across 8 cores (BC=4 batch elements/core).

v2 design (per core):
  - wk in fp8e4 (x64 scaled), single j-major DMA; coeff = lm8 @ wk8 on PE,
    psum -> kernT_sb fp16 via DVE tensor_scalar (x 1/64).  kh ordering
    hc*4096 + k*128 + h so the hc0 half finishes first; relayout via a
    64KB DRAM bounce into (32 k, 2 hc, 4 b, 128 h) conv weights.
  - conv: per (b,hc): 1 LDW + 8 matmuls (K=32, N=512) into (128,1024)
    psum tiles; DVE adds enc (fp16) -> hid fp16; one big ACT tanh
    (128,4096) with per-partition glob bias -> tan fp16.
  - score: ws stationary (P=1), tan streamed N=512, M=1 outputs packed
    4-wide into psum rows {0,32,64,96} via tile_position col groups;
    mask/bs added with one (4,512)-rhs matmul; ACT exp with accum_out
    gives row sums; denominators via selector matmul; normalize on GPSIMD.
"""

import os
import sys

import numpy as np

if "/opt/trn_rl_repo" not in sys.path:
    sys.path.insert(0, "/opt/trn_rl_repo")

import ml_dtypes

T, B, H, L, K = 4096, 32, 256, 512, 32
NCORES = 8
BC = B // NCORES          # 4 batches per core
HCHUNKS = H // 128        # 2
TTILE = 512
NTT = T // TTILE          # 8 t-tiles per (b, hc)
WK_SCALE = 64.0

_CACHE = {}


def _build_program():
    import concourse.bacc as bacc
    import concourse.bass as bass
    import concourse.mybir as mybir
    import concourse.tile as tile
    from contextlib import ExitStack

    dt = mybir.dt
    fp32 = dt.float32
    bf16 = dt.bfloat16
    fp16 = dt.float16
    fp8 = dt.float8e4
    ts = bass.ts

    nc = bacc.Bacc(
        "TRN2",
        target_bir_lowering=False,
        debug=False,
        enable_asserts=False,
        num_devices=NCORES,
    )

    enc = nc.dram_tensor("enc", (BC, HCHUNKS, 128, T), fp16, kind="ExternalInput").ap()
    win = nc.dram_tensor("win", (BC, K, T), fp16, kind="ExternalInput").ap()
    wk8 = nc.dram_tensor("wk8", (128, 16, 4, 512), fp8, kind="ExternalInput").ap()
    lm8 = nc.dram_tensor("lm8", (128, 4, BC), fp8, kind="ExternalInput").ap()
    lmb = nc.dram_tensor("lmb", (128, 4, BC), bf16, kind="ExternalInput").ap()
    wgt = nc.dram_tensor("wgt", (128, 4, H), bf16, kind="ExternalInput").ap()
    bgp = nc.dram_tensor("bgp", (1, H), bf16, kind="ExternalInput").ap()
    ws2 = nc.dram_tensor("ws2", (128, HCHUNKS), fp16, kind="ExternalInput").ap()
    mkc = nc.dram_tensor("mkc", (4, 2 * BC, TTILE), fp8, kind="ExternalInput").ap()
    sel4 = nc.dram_tensor("sel4", (4, 128), fp8, kind="ExternalInput").ap()
    selr = nc.dram_tensor("selr", (128, 1), fp32, kind="ExternalInput").ap()
    att = nc.dram_tensor("att", (4, 2 * BC, TTILE), fp16, kind="ExternalOutput").ap()

    TanhF = mybir.ActivationFunctionType.Tanh
    ExpF = mybir.ActivationFunctionType.Exp
    Add = mybir.AluOpType.add

    with tile.TileContext(nc) as tc, ExitStack() as ctx:
        # ---------- pools ----------
        small_pool = ctx.enter_context(tc.tile_pool(name="small", bufs=1))
        big_sb = ctx.enter_context(tc.tile_pool(name="bigsb", bufs=1))
        wk_pool = ctx.enter_context(tc.tile_pool(name="wkp", bufs=16))
        kern_pool = ctx.enter_context(tc.tile_pool(name="kernp", bufs=1))
        hid_pool = ctx.enter_context(tc.tile_pool(name="hidp", bufs=4))
        tan_pool = ctx.enter_context(tc.tile_pool(name="tanp", bufs=4))
        psum_big = ctx.enter_context(tc.tile_pool(name="psumb", bufs=3, space="PSUM"))
        psum_sp = ctx.enter_context(tc.tile_pool(name="psums", bufs=2, space="PSUM"))
        dram_pool = ctx.enter_context(tc.tile_pool(name="dramp", bufs=1, space="DRAM"))

        # ---------- small input loads (sync queue, before enc/win) ----------
        lm8_sb = small_pool.tile([128, 4, BC], fp8)
        nc.sync.dma_start(lm8_sb[:], lm8)
        lmb_sb = small_pool.tile([128, 4, BC], bf16)
        nc.sync.dma_start(lmb_sb[:], lmb)
        wgt_sb = small_pool.tile([128, 4, H], bf16)
        nc.sync.dma_start(wgt_sb[:], wgt)
        bg_sb = small_pool.tile([1, H], bf16)
        nc.sync.dma_start(bg_sb[:], bgp)
        ws_sb = small_pool.tile([128, HCHUNKS], fp16)
        nc.sync.dma_start(ws_sb[:], ws2)
        msk_sb = small_pool.tile([4, 2 * BC, TTILE], fp8)
        nc.sync.dma_start(msk_sb[:], mkc)
        sel4_sb = small_pool.tile([4, 128], fp8)
        nc.sync.dma_start(sel4_sb[:], sel4)
        selr_sb = small_pool.tile([128, 1], fp32)
        nc.sync.dma_start(selr_sb[:], selr)

        one_b = small_pool.tile([1, BC], bf16)
        nc.vector.memset(one_b[:], 1.0)
        ones1x128 = small_pool.tile([1, 128], fp32)
        nc.vector.memset(ones1x128[:], 1.0)

        # PE warmup filler: keep HAM busy from t~1.5us until the wk stream
        # arrives (~10.5us) so the coeff phase runs at 2.4 GHz
        warm_sb = small_pool.tile([128, 128], bf16)
        nc.vector.memset(warm_sb[:], 0.0)
        wps = psum_sp.tile([64, 128], fp32, tag="sp", name="wps")
        for _ in range(100):
            nc.tensor.matmul(
                wps[:], warm_sb[:, 0:64], warm_sb[:], start=True, stop=True
            )


        # ---------- big input loads ----------
        # wk8 j-major in 16 small chunks: PE never starves, HAM warms early
        wk_tiles = []
        for jc in range(16):
            wkt = wk_pool.tile([128, 1, 4, 512], fp8, tag="wk", name=f"wk{jc}")
            nc.gpsimd.dma_start(wkt[:], wk8[:, jc : jc + 1, :, :])
            wk_tiles.append(wkt)
        # win: all 4 b packed at rows {0,32,64,96}; on the priority queue
        winA = big_sb.tile([128, T], fp16)
        for b in range(BC):
            nc.gpsimd.dma_start(winA[32 * b : 32 * b + 32, :], win[b, :, :])
        enc_sb = big_sb.tile([128, BC, HCHUNKS, T], fp16)
        # gpsimd: first 4 units' enc; sync: late units (after wk/win clear)
        enc_order = [(0, 0), (0, 1), (1, 0), (1, 1), (2, 0), (2, 1), (3, 0), (3, 1)]
        for n, (b, hc) in enumerate(enc_order):
            nc.gpsimd.dma_start(enc_sb[:, b, hc, :], enc[b, hc, :, :])

        # ---------- persistent sbuf ----------
        scr = dram_pool.tile([BC, K * H], fp16)
        coef_sb = big_sb.tile([128, HCHUNKS, 128], fp16)  # [32*b + k, hc, h]
        exp_sb = big_sb.tile([128, 2 * BC, TTILE], fp16)
        acc_sb = small_pool.tile([128, 2 * BC], fp32)
        att_sb = big_sb.tile([128, 2 * BC, TTILE], fp16)
        glob_sb = small_pool.tile([128, HCHUNKS, BC], fp32)
        dsum_sb = small_pool.tile([1, 2 * BC], fp32)
        den_sb = small_pool.tile([1, BC], fp32)
        rec_sb = small_pool.tile([1, BC], fp32)
        recb_sb = small_pool.tile([128, BC], fp32)

        def coeff_chunk(jlist, kernT_sb):
            for j in jlist:
                cps = psum_big.tile([BC, 512], fp32, tag="conv", name="cps")
                for lc in range(4):
                    nc.tensor.matmul(
                        cps[:],
                        lm8_sb[:, lc, :],
                        wk_tiles[j][:, 0, lc, :],
                        start=(lc == 0),
                        stop=(lc == 3),
                    )
                nc.vector.tensor_scalar_mul(
                    kernT_sb[:, ts(j % 8, 512)], cps[:], 1.0 / WK_SCALE
                )

        def relayout(hcc, kernT_sb):
            nc.sync.dma_start(scr[:, ts(hcc, 4096)], kernT_sb[:])
            for b in range(BC):
                nc.sync.dma_start(
                    coef_sb[32 * b : 32 * b + 32, hcc, :],
                    scr[b, ts(hcc, 4096)].rearrange("(k h) -> k h", k=K),
                )

        def glob_phase():
            for hc in range(HCHUNKS):
                gps = psum_sp.tile([128, BC], fp32, tag="sp", name="gps")
                for lc in range(4):
                    nc.tensor.matmul(
                        gps[:],
                        wgt_sb[:, lc, ts(hc, 128)],
                        lmb_sb[:, lc, :],
                        start=(lc == 0),
                        stop=False,
                    )
                nc.tensor.matmul(
                    gps[:], bg_sb[:, ts(hc, 128)], one_b[:], start=False, stop=True
                )
                nc.scalar.copy(glob_sb[:, hc, :], gps[:])

        def conv_unit(b, hc):
            """conv + enc add + tanh for one (b, hc); returns tan tile."""
            lhsT = coef_sb[32 * b : 32 * b + 32, hc, :]
            base = 32 * b
            hid = hid_pool.tile([128, T], fp16, tag="hid")
            for th in range(4):
                cpsum = psum_big.tile([128, 1024], fp32, tag="conv", name="cpsum")
                for q in range(2):
                    tt = th * 2 + q
                    nc.tensor.matmul(
                        cpsum[:, ts(q, 512)],
                        lhsT,
                        winA[base : base + 32, ts(tt, 512)],
                        start=True,
                        stop=True,
                        tile_position=(base, 0),
                    )
                nc.vector.tensor_tensor(
                    hid[:, ts(th, 1024)],
                    cpsum[:],
                    enc_sb[:, b, hc, ts(th, 1024)],
                    Add,
                )
            tan = tan_pool.tile([128, T], fp16, tag="tan")
            nc.scalar.activation(
                tan[:], hid[:], TanhF, bias=glob_sb[:, hc, b : b + 1], scale=1.0
            )
            return tan

        def score_unit(b, tans):
            """score matmuls + exp for both s halves of batch b."""
            for s in range(2):
                sp = psum_sp.tile([128, TTILE], fp32, tag="sp", name="sp")
                nc.tensor.matmul(
                    sp[:],
                    sel4_sb[:],
                    msk_sb[:, s * BC + b, :],
                    start=True,
                    stop=False,
                    skip_group_check=True,
                )
                for j in range(4):
                    for hc in range(HCHUNKS):
                        nc.tensor.matmul(
                            sp[32 * j : 32 * j + 1, :],
                            ws_sb[:, hc : hc + 1],
                            tans[hc][:, ts(4 * s + j, 512)],
                            start=False,
                            stop=(j == 3 and hc == HCHUNKS - 1),
                            tile_position=(0, 32 * j),
                            skip_group_check=True,
                        )
                col = s * BC + b
                nc.scalar.activation(
                    exp_sb[:, col, :],
                    sp[:],
                    ExpF,
                    bias=0.0,
                    scale=1.0,
                    accum_out=acc_sb[:, col : col + 1],
                )

        def softmax_tail():
            dps = psum_sp.tile([1, 2 * BC], fp32, tag="sp", name="dps")
            nc.tensor.matmul(dps[:], selr_sb[:], acc_sb[:], start=True, stop=True)
            nc.scalar.copy(dsum_sb[:], dps[:])
            nc.vector.tensor_tensor(
                den_sb[:], dsum_sb[:, 0:BC], dsum_sb[:, BC : 2 * BC], Add
            )
            nc.vector.reciprocal(rec_sb[:], den_sb[:])
            bps = psum_sp.tile([128, BC], fp32, tag="sp", name="bps")
            nc.tensor.matmul(bps[:], ones1x128[:], rec_sb[:], start=True, stop=True)
            nc.scalar.copy(recb_sb[:], bps[:])
            for b in range(BC):
                nc.scalar.mul(
                    att_sb[:, b, :], exp_sb[:, b, :], recb_sb[:, b : b + 1]
                )
                nc.vector.tensor_scalar_mul(
                    att_sb[:, BC + b, :], exp_sb[:, BC + b, :], recb_sb[:, b : b + 1]
                )

        # ---- emission order ----
        kernT0 = kern_pool.tile([BC, 8 * 512], fp16, tag="kern", name="kernT0")
        coeff_chunk(range(0, 8), kernT0)
        relayout(0, kernT0)
        glob_phase()

        kernT1 = kern_pool.tile([BC, 8 * 512], fp16, tag="kern", name="kernT1")
        coeff_chunk(range(8, 16), kernT1)
        relayout(1, kernT1)

        for b in range(BC):
            t0 = conv_unit(b, 0)
            t1 = conv_unit(b, 1)
            score_unit(b, [t0, t1])
        softmax_tail()

        for j in range(4):
            nc.scalar.dma_start(att[j : j + 1], att_sb[32 * j : 32 * j + 1, :, :])

    nc.compile()
    return nc


def _get_program():
    if "nc" not in _CACHE:
        _CACHE["nc"] = _build_program()
    return _CACHE["nc"]


def _prep_inputs(encoded_contribution, mask, lm_state, prev_att_weights,
                 Wk, bk, Wg, bg, Ws, bs):
    """Host-side shard + layout prep. Returns list of per-core input dicts."""
    import concourse.mybir as mybir

    f32 = np.float32
    bf16 = ml_dtypes.bfloat16
    f8 = mybir.dt.np(mybir.dt.float8e4)

    enc = np.asarray(encoded_contribution, dtype=f32)
    mask = np.asarray(mask, dtype=f32)
    lm = np.asarray(lm_state, dtype=f32)
    prev = np.asarray(prev_att_weights, dtype=f32)
    Wk = np.asarray(Wk, dtype=f32)
    bk = np.asarray(bk, dtype=f32)
    Wg = np.asarray(Wg, dtype=f32)
    bg = np.asarray(bg, dtype=f32)
    Ws = np.asarray(Ws, dtype=f32)
    bs = np.asarray(bs, dtype=f32)

    # toeplitz windows: win[b, k, t] = prev_pad[b, k + t]
    prev_pad = np.zeros((B, T + K - 1), dtype=f32)
    prev_pad[:, K - 1 :] = prev.T
    win_f32 = np.lib.stride_tricks.sliding_window_view(prev_pad, T, axis=1)

    # fold the conv bias bk into enc: contribution = sum_k win[b,k,t]*bk[h,k]
    if np.any(bk):
        enc = enc + np.einsum(
            "bkt,hk->tbh", win_f32, bk.reshape(H, K), optimize=True
        )

    # enc: (T, B, H) -> (B, H, T) -> (NCORES, BC, HCHUNKS, 128, T) fp16
    enc_t = np.ascontiguousarray(enc.transpose(1, 2, 0).astype(np.float16)).reshape(
        NCORES, BC, HCHUNKS, 128, T
    )
    win_full = win_f32.astype(np.float16).reshape(NCORES, BC, K, T)

    # WkP64[l, kh'] with kh' = hc*4096 + k*128 + h  (Wk row = (hc*128+h)*32 + k)
    wkp = (
        Wk.reshape(HCHUNKS, 128, K, L)       # (hc, h, k, l)
        .transpose(3, 0, 2, 1)               # (l, hc, k, h)
        .reshape(L, K * H)
        * WK_SCALE
    ).astype(f8)
    # dram layout (128 lp, 16 j, 4 lc, 512 c): [lc*128+lp, j*512+c]
    wk8 = np.ascontiguousarray(
        wkp.reshape(4, 128, 16, 512).transpose(1, 2, 0, 3)
    )


    # lm chunks: (128, 4, B)
    lmT = np.ascontiguousarray(lm.T.reshape(4, 128, B).transpose(1, 0, 2))

    # WgT chunks: (128, 4, H)
    wgt = np.ascontiguousarray(Wg.T.reshape(4, 128, H).transpose(1, 0, 2)).astype(bf16)

    bgp = np.ascontiguousarray(bg.reshape(1, H)).astype(bf16)
    ws2 = np.ascontiguousarray(Ws[0].reshape(HCHUNKS, 128).T).astype(np.float16)

    # selector constants
    sel4 = np.zeros((4, 128), dtype=f8)
    for p in range(4):
        sel4[p, 32 * p] = 1.0
    selr = np.zeros((128, 1), dtype=f32)
    selr[::32, 0] = 1.0

    in_maps = []
    for c in range(NCORES):
        m = mask[:, c * BC : (c + 1) * BC] + bs[0]   # (T, BC)
        # mkc[j, b*2+s, c] = m[(4s+j)*512 + c, b]
        mr = m.reshape(2, 4, TTILE, BC)              # (s, j, cc, b)
        mkc = np.ascontiguousarray(
            np.clip(mr.transpose(1, 0, 3, 2).reshape(4, 2 * BC, TTILE), -440.0, 440.0)
        ).astype(f8)
        lmc = np.ascontiguousarray(lmT[:, :, c * BC : (c + 1) * BC])
        in_maps.append(
            {
                "enc": np.ascontiguousarray(enc_t[c]),
                "win": np.ascontiguousarray(win_full[c]),
                "wk8": wk8,
                "lm8": lmc.astype(f8),
                "lmb": lmc.astype(bf16),
                "wgt": wgt,
                "bgp": bgp,
                "ws2": ws2,
                "mkc": mkc,
                "sel4": sel4,
                "selr": selr,
            }
        )
    return in_maps


def _assemble_output(per_core):
    out = np.empty((T, B), dtype=np.float32)
    for c in range(NCORES):
        A = np.asarray(per_core[c], dtype=np.float32)   # (4, 2*BC, 512)
        # A[j, s*BC+b, cc] = att[t=(4s+j)*512+cc, c*BC+b]
        blk = A.reshape(4, 2, BC, TTILE).transpose(1, 0, 3, 2).reshape(T, BC)
        out[:, c * BC : (c + 1) * BC] = blk
    return out


def kernel(**inputs):
    from concourse.bass_utils import run_bass_kernel_spmd

    in_maps = _prep_inputs(**inputs)
    nc = _get_program()
    trace = bool(os.environ.get("BASS_TRACE"))
    res = run_bass_kernel_spmd(nc, in_maps, list(range(NCORES)), trace=trace)
    _CACHE["last_results"] = res
    return _assemble_output([r["att"] for r in res.results])


# revision 35
# speedup vs baseline: 1.0748x; 1.0168x over previous
"""Trainium2 Bass kernel for nn_LocalAttention (T=4096, B=32, H=256, L=512, K=32).

Sharding: data-parallel over batch B across 8 cores (BC=4 batch elements/core).

v2 design (per core):
  - wk in fp8e4 (x64 scaled), single j-major DMA; coeff = lm8 @ wk8 on PE,
    psum -> kernT_sb fp16 via DVE tensor_scalar (x 1/64).  kh ordering
    hc*4096 + k*128 + h so the hc0 half finishes first; relayout via a
    64KB DRAM bounce into (32 k, 2 hc, 4 b, 128 h) conv weights.
  - conv: per (b,hc): 1 LDW + 8 matmuls (K=32, N=512) into (128,1024)
    psum tiles; DVE adds enc (fp16) -> hid fp16; one big ACT tanh
    (128,4096) with per-partition glob bias -> tan fp16.
  - score: ws stationary (P=1), tan streamed N=512, M=1 outputs packed
    4-wide into psum rows {0,32,64,96} via tile_position col groups;
    mask/bs added with one (4,512)-rhs matmul; ACT exp with accum_out
    gives row sums; denominators via selector matmul; normalize on GPSIMD.
"""

import os
import sys

import numpy as np

if "/opt/trn_rl_repo" not in sys.path:
    sys.path.insert(0, "/opt/trn_rl_repo")

import ml_dtypes

T, B, H, L, K = 4096, 32, 256, 512, 32
NCORES = 8
BC = B // NCORES          # 4 batches per core
HCHUNKS = H // 128        # 2
TTILE = 512
NTT = T // TTILE          # 8 t-tiles per (b, hc)
WK_SCALE = 64.0

_CACHE = {}


def _build_program():
    import concourse.bacc as bacc
    import concourse.bass as bass
    import concourse.mybir as mybir
    import concourse.tile as tile
    from contextlib import ExitStack

    dt = mybir.dt
    fp32 = dt.float32
    bf16 = dt.bfloat16
    fp16 = dt.float16
    fp8 = dt.float8e4
    ts = bass.ts

    nc = bacc.Bacc(
        "TRN2",
        target_bir_lowering=False,
        debug=False,
        enable_asserts=False,
        num_devices=NCORES,
    )

    enc = nc.dram_tensor("enc", (BC, HCHUNKS, 128, T), fp16, kind="ExternalInput").ap()
    win = nc.dram_tensor("win", (BC, K, T), fp16, kind="ExternalInput").ap()
    wk8 = nc.dram_tensor("wk8", (128, 16, 4, 512), fp8, kind="ExternalInput").ap()
    lm8 = nc.dram_tensor("lm8", (128, 4, BC), fp8, kind="ExternalInput").ap()
    lmb = nc.dram_tensor("lmb", (128, 4, BC), bf16, kind="ExternalInput").ap()
    wgt = nc.dram_tensor("wgt", (128, 4, H), bf16, kind="ExternalInput").ap()
    bgp = nc.dram_tensor("bgp", (1, H), bf16, kind="ExternalInput").ap()
    ws2 = nc.dram_tensor("ws2", (128, HCHUNKS), fp16, kind="ExternalInput").ap()
    mkc = nc.dram_tensor("mkc", (4, 2 * BC, TTILE), fp8, kind="ExternalInput").ap()
    sel4 = nc.dram_tensor("sel4", (4, 128), fp8, kind="ExternalInput").ap()
    selr = nc.dram_tensor("selr", (128, 1), fp32, kind="ExternalInput").ap()
    att = nc.dram_tensor("att", (4, 2 * BC, TTILE), fp16, kind="ExternalOutput").ap()

    TanhF = mybir.ActivationFunctionType.Tanh
    ExpF = mybir.ActivationFunctionType.Exp
    Add = mybir.AluOpType.add

    with tile.TileContext(nc) as tc, ExitStack() as ctx:
        # ---------- pools ----------
        small_pool = ctx.enter_context(tc.tile_pool(name="small", bufs=1))
        big_sb = ctx.enter_context(tc.tile_pool(name="bigsb", bufs=1))
        wk_pool = ctx.enter_context(tc.tile_pool(name="wkp", bufs=16))
        kern_pool = ctx.enter_context(tc.tile_pool(name="kernp", bufs=1))
        hid_pool = ctx.enter_context(tc.tile_pool(name="hidp", bufs=3))
        tan_pool = ctx.enter_context(tc.tile_pool(name="tanp", bufs=3))
        psum_big = ctx.enter_context(tc.tile_pool(name="psumb", bufs=3, space="PSUM"))
        psum_sp = ctx.enter_context(tc.tile_pool(name="psums", bufs=2, space="PSUM"))
        dram_pool = ctx.enter_context(tc.tile_pool(name="dramp", bufs=1, space="DRAM"))

        # ---------- small input loads (sync queue, before enc/win) ----------
        lm8_sb = small_pool.tile([128, 4, BC], fp8)
        nc.sync.dma_start(lm8_sb[:], lm8)
        lmb_sb = small_pool.tile([128, 4, BC], bf16)
        nc.sync.dma_start(lmb_sb[:], lmb)
        wgt_sb = small_pool.tile([128, 4, H], bf16)
        nc.sync.dma_start(wgt_sb[:], wgt)
        bg_sb = small_pool.tile([1, H], bf16)
        nc.sync.dma_start(bg_sb[:], bgp)
        ws_sb = small_pool.tile([128, HCHUNKS], fp16)
        nc.sync.dma_start(ws_sb[:], ws2)
        msk_sb = small_pool.tile([4, 2 * BC, TTILE], fp8)
        nc.sync.dma_start(msk_sb[:], mkc)
        sel4_sb = small_pool.tile([4, 128], fp8)
        nc.sync.dma_start(sel4_sb[:], sel4)
        selr_sb = small_pool.tile([128, 1], fp32)
        nc.sync.dma_start(selr_sb[:], selr)

        one_b = small_pool.tile([1, BC], bf16)
        nc.vector.memset(one_b[:], 1.0)
        ones1x128 = small_pool.tile([1, 128], fp32)
        nc.vector.memset(ones1x128[:], 1.0)

        # PE warmup filler: keep HAM busy from t~1.5us until the wk stream
        # arrives (~10.5us) so the coeff phase runs at 2.4 GHz
        warm_sb = small_pool.tile([128, 128], bf16)
        nc.vector.memset(warm_sb[:], 0.0)
        wps = psum_sp.tile([64, 128], fp32, tag="sp", name="wps")
        for _ in range(120):
            nc.tensor.matmul(
                wps[:], warm_sb[:, 0:64], warm_sb[:], start=True, stop=True
            )


        # ---------- big input loads ----------
        # wk8 j-major in 16 small chunks: PE never starves, HAM warms early
        wk_tiles = []
        for jc in range(16):
            wkt = wk_pool.tile([128, 1, 4, 512], fp8, tag="wk", name=f"wk{jc}")
            nc.gpsimd.dma_start(wkt[:], wk8[:, jc : jc + 1, :, :])
            wk_tiles.append(wkt)
        # win: all 4 b packed at rows {0,32,64,96}; on the priority queue
        winA = big_sb.tile([128, T], fp16)
        for b in range(BC):
            nc.gpsimd.dma_start(winA[32 * b : 32 * b + 32, :], win[b, :, :])
        enc_sb = big_sb.tile([128, BC, HCHUNKS, T], fp16)
        # gpsimd: first 4 units' enc; sync: late units (after wk/win clear)
        enc_order = [(0, 0), (0, 1), (1, 0), (1, 1), (2, 0), (2, 1), (3, 0), (3, 1)]
        for n, (b, hc) in enumerate(enc_order):
            nc.gpsimd.dma_start(enc_sb[:, b, hc, :], enc[b, hc, :, :])

        # ---------- persistent sbuf ----------
        scr = dram_pool.tile([BC, K * H], fp16)
        coef_sb = big_sb.tile([128, HCHUNKS, 128], fp16)  # [32*b + k, hc, h]
        exp_sb = big_sb.tile([128, 2 * BC, TTILE], fp16)
        acc_sb = small_pool.tile([128, 2 * BC], fp32)
        att_sb = big_sb.tile([128, 2 * BC, TTILE], fp16)
        glob_sb = small_pool.tile([128, HCHUNKS, BC], fp32)
        dsum_sb = small_pool.tile([1, 2 * BC], fp32)
        den_sb = small_pool.tile([1, BC], fp32)
        rec_sb = small_pool.tile([1, BC], fp32)
        recb_sb = small_pool.tile([128, BC], fp32)

        def coeff_chunk(jlist, kernT_sb):
            for j in jlist:
                cps = psum_big.tile([BC, 512], fp32, tag="conv", name="cps")
                for lc in range(4):
                    nc.tensor.matmul(
                        cps[:],
                        lm8_sb[:, lc, :],
                        wk_tiles[j][:, 0, lc, :],
                        start=(lc == 0),
                        stop=(lc == 3),
                    )
                nc.vector.tensor_scalar_mul(
                    kernT_sb[:, ts(j % 8, 512)], cps[:], 1.0 / WK_SCALE
                )

        def relayout(hcc, kernT_sb):
            nc.sync.dma_start(scr[:, ts(hcc, 4096)], kernT_sb[:])
            for b in range(BC):
                nc.sync.dma_start(
                    coef_sb[32 * b : 32 * b + 32, hcc, :],
                    scr[b, ts(hcc, 4096)].rearrange("(k h) -> k h", k=K),
                )

        def glob_phase():
            for hc in range(HCHUNKS):
                gps = psum_sp.tile([128, BC], fp32, tag="sp", name="gps")
                for lc in range(4):
                    nc.tensor.matmul(
                        gps[:],
                        wgt_sb[:, lc, ts(hc, 128)],
                        lmb_sb[:, lc, :],
                        start=(lc == 0),
                        stop=False,
                    )
                nc.tensor.matmul(
                    gps[:], bg_sb[:, ts(hc, 128)], one_b[:], start=False, stop=True
                )
                nc.scalar.copy(glob_sb[:, hc, :], gps[:])

        def conv_unit(b, hc):
            """conv + enc add + tanh for one (b, hc); returns tan tile."""
            lhsT = coef_sb[32 * b : 32 * b + 32, hc, :]
            base = 32 * b
            hid = hid_pool.tile([128, T], fp16, tag="hid")
            for th in range(4):
                cpsum = psum_big.tile([128, 1024], fp32, tag="conv", name="cpsum")
                for q in range(2):
                    tt = th * 2 + q
                    nc.tensor.matmul(
                        cpsum[:, ts(q, 512)],
                        lhsT,
                        winA[base : base + 32, ts(tt, 512)],
                        start=True,
                        stop=True,
                        tile_position=(base, 0),
                    )
                nc.vector.tensor_tensor(
                    hid[:, ts(th, 1024)],
                    cpsum[:],
                    enc_sb[:, b, hc, ts(th, 1024)],
                    Add,
                )
            tan = tan_pool.tile([128, T], fp16, tag="tan")
            nc.scalar.activation(
                tan[:], hid[:], TanhF, bias=glob_sb[:, hc, b : b + 1], scale=1.0
            )
            return tan

        def score_unit(b, tans):
            """score matmuls + exp for both s halves of batch b."""
            for s in range(2):
                sp = psum_sp.tile([128, TTILE], fp32, tag="sp", name="sp")
                nc.tensor.matmul(
                    sp[:],
                    sel4_sb[:],
                    msk_sb[:, s * BC + b, :],
                    start=True,
                    stop=False,
                    skip_group_check=True,
                )
                for j in range(4):
                    for hc in range(HCHUNKS):
                        nc.tensor.matmul(
                            sp[32 * j : 32 * j + 1, :],
                            ws_sb[:, hc : hc + 1],
                            tans[hc][:, ts(4 * s + j, 512)],
                            start=False,
                            stop=(j == 3 and hc == HCHUNKS - 1),
                            tile_position=(0, 32 * j),
                            skip_group_check=True,
                        )
                col = s * BC + b
                nc.scalar.activation(
                    exp_sb[:, col, :],
                    sp[:],
                    ExpF,
                    bias=0.0,
                    scale=1.0,
                    accum_out=acc_sb[:, col : col + 1],
                )

        def softmax_tail():
            dps = psum_sp.tile([1, 2 * BC], fp32, tag="sp", name="dps")
            nc.tensor.matmul(dps[:], selr_sb[:], acc_sb[:], start=True, stop=True)
            nc.scalar.copy(dsum_sb[:], dps[:])
            nc.vector.tensor_tensor(
                den_sb[:], dsum_sb[:, 0:BC], dsum_sb[:, BC : 2 * BC], Add
            )
            nc.vector.reciprocal(rec_sb[:], den_sb[:])
            bps = psum_sp.tile([128, BC], fp32, tag="sp", name="bps")
            nc.tensor.matmul(bps[:], ones1x128[:], rec_sb[:], start=True, stop=True)
            nc.scalar.copy(recb_sb[:], bps[:])
            for b in range(BC):
                nc.scalar.mul(
                    att_sb[:, b, :], exp_sb[:, b, :], recb_sb[:, b : b + 1]
                )
                nc.vector.tensor_scalar_mul(
                    att_sb[:, BC + b, :], exp_sb[:, BC + b, :], recb_sb[:, b : b + 1]
                )

        # ---- emission order ----
        kernT0 = kern_pool.tile([BC, 8 * 512], fp16, tag="kern", name="kernT0")
        coeff_chunk(range(0, 8), kernT0)
        relayout(0, kernT0)
        glob_phase()

        kernT1 = kern_pool.tile([BC, 8 * 512], fp16, tag="kern", name="kernT1")
        coeff_chunk(range(8, 16), kernT1)
        relayout(1, kernT1)

        for b in range(BC):
            t0 = conv_unit(b, 0)
            t1 = conv_unit(b, 1)
            score_unit(b, [t0, t1])
        softmax_tail()

        for j in range(4):
            nc.scalar.dma_start(att[j : j + 1], att_sb[32 * j : 32 * j + 1, :, :])

    nc.compile()
    return nc


def _get_program():
    if "nc" not in _CACHE:
        _CACHE["nc"] = _build_program()
    return _CACHE["nc"]


def _prep_inputs(encoded_contribution, mask, lm_state, prev_att_weights,
                 Wk, bk, Wg, bg, Ws, bs):
    """Host-side shard + layout prep. Returns list of per-core input dicts."""
    import concourse.mybir as mybir

    f32 = np.float32
    bf16 = ml_dtypes.bfloat16
    f8 = mybir.dt.np(mybir.dt.float8e4)

    enc = np.asarray(encoded_contribution, dtype=f32)
    mask = np.asarray(mask, dtype=f32)
    lm = np.asarray(lm_state, dtype=f32)
    prev = np.asarray(prev_att_weights, dtype=f32)
    Wk = np.asarray(Wk, dtype=f32)
    bk = np.asarray(bk, dtype=f32)
    Wg = np.asarray(Wg, dtype=f32)
    bg = np.asarray(bg, dtype=f32)
    Ws = np.asarray(Ws, dtype=f32)
    bs = np.asarray(bs, dtype=f32)

    # toeplitz windows: win[b, k, t] = prev_pad[b, k + t]
    prev_pad = np.zeros((B, T + K - 1), dtype=f32)
    prev_pad[:, K - 1 :] = prev.T
    win_f32 = np.lib.stride_tricks.sliding_window_view(prev_pad, T, axis=1)

    # fold the conv bias bk into enc: contribution = sum_k win[b,k,t]*bk[h,k]
    if np.any(bk):
        enc = enc + np.einsum(
            "bkt,hk->tbh", win_f32, bk.reshape(H, K), optimize=True
        )

    # enc: (T, B, H) -> (B, H, T) -> (NCORES, BC, HCHUNKS, 128, T) fp16
    enc_t = np.ascontiguousarray(enc.transpose(1, 2, 0).astype(np.float16)).reshape(
        NCORES, BC, HCHUNKS, 128, T
    )
    win_full = win_f32.astype(np.float16).reshape(NCORES, BC, K, T)

    # WkP64[l, kh'] with kh' = hc*4096 + k*128 + h  (Wk row = (hc*128+h)*32 + k)
    wkp = (
        Wk.reshape(HCHUNKS, 128, K, L)       # (hc, h, k, l)
        .transpose(3, 0, 2, 1)               # (l, hc, k, h)
        .reshape(L, K * H)
        * WK_SCALE
    ).astype(f8)
    # dram layout (128 lp, 16 j, 4 lc, 512 c): [lc*128+lp, j*512+c]
    wk8 = np.ascontiguousarray(
        wkp.reshape(4, 128, 16, 512).transpose(1, 2, 0, 3)
    )


    # lm chunks: (128, 4, B)
    lmT = np.ascontiguousarray(lm.T.reshape(4, 128, B).transpose(1, 0, 2))

    # WgT chunks: (128, 4, H)
    wgt = np.ascontiguousarray(Wg.T.reshape(4, 128, H).transpose(1, 0, 2)).astype(bf16)

    bgp = np.ascontiguousarray(bg.reshape(1, H)).astype(bf16)
    ws2 = np.ascontiguousarray(Ws[0].reshape(HCHUNKS, 128).T).astype(np.float16)

    # selector constants
    sel4 = np.zeros((4, 128), dtype=f8)
    for p in range(4):
        sel4[p, 32 * p] = 1.0
    selr = np.zeros((128, 1), dtype=f32)
    selr[::32, 0] = 1.0

    in_maps = []
    for c in range(NCORES):
        m = mask[:, c * BC : (c + 1) * BC] + bs[0]   # (T, BC)
        # mkc[j, b*2+s, c] = m[(4s+j)*512 + c, b]
        mr = m.reshape(2, 4, TTILE, BC)              # (s, j, cc, b)
        mkc = np.ascontiguousarray(
            np.clip(mr.transpose(1, 0, 3, 2).reshape(4, 2 * BC, TTILE), -440.0, 440.0)
        ).astype(f8)
        lmc = np.ascontiguousarray(lmT[:, :, c * BC : (c + 1) * BC])
        in_maps.append(
            {
                "enc": np.ascontiguousarray(enc_t[c]),
                "win": np.ascontiguousarray(win_full[c]),
                "wk8": wk8,
                "lm8": lmc.astype(f8),
                "lmb": lmc.astype(bf16),
                "wgt": wgt,
                "bgp": bgp,
                "ws2": ws2,
                "mkc": mkc,
                "sel4": sel4,
                "selr": selr,
            }
        )
    return in_maps


def _assemble_output(per_core):
    out = np.empty((T, B), dtype=np.float32)
    for c in range(NCORES):
        A = np.asarray(per_core[c], dtype=np.float32)   # (4, 2*BC, 512)
        # A[j, s*BC+b, cc] = att[t=(4s+j)*512+cc, c*BC+b]
        blk = A.reshape(4, 2, BC, TTILE).transpose(1, 0, 3, 2).reshape(T, BC)
        out[:, c * BC : (c + 1) * BC] = blk
    return out


def kernel(**inputs):
    from concourse.bass_utils import run_bass_kernel_spmd

    in_maps = _prep_inputs(**inputs)
    nc = _get_program()
    trace = bool(os.environ.get("BASS_TRACE"))
    res = run_bass_kernel_spmd(nc, in_maps, list(range(NCORES)), trace=trace)
    _CACHE["last_results"] = res
    return _assemble_output([r["att"] for r in res.results])
